# revision 1
# baseline (speedup 1.0000x reference)
"""Trainium2 Bass kernel for nn_BasicAttentionModel (3-layer GAT + edge MLP).

Strategy (8-core SPMD, dst-partitioned edge processing):
  - Nodes padded to NP and split into regular 128-node tiles; each core owns
    TPC consecutive tiles (its dst range).  Edges (with self-loops) are
    dst-sorted into those tiles; inside a tile they are grouped into SUBS
    sub-tiles by src chunk (so int16 gather indices stay in range), padded
    with index 0 (harmless: masked out by the selection matrix).
  - Per layer, a replicated node phase computes a 256B-row node table
    [prev+b | al_s | al_d] with one matmul; the edge phase gathers rows by
    src via dma_gather (256B rows, 4 SWDGE queues), expands h = prev @ W on
    the PE per 128-edge group, forms attention weights (no-max softmax is
    numerically exact here), and reduces per-dst-segment with a selection
    matrix matmul into PSUM.
  - Edge MLP runs the same tiling; u/v tables are gathered/expanded and the
    3-layer MLP runs on PE per group.  The host only reorders/concats
    arrays between the four launches (no float math on host).
"""
import numpy as np
import ml_dtypes

import concourse.bacc as bacc
import concourse.bass as bass
import concourse.mybir as mybir
import concourse.tile as tile
from concourse.bass_utils import run_bass_kernel_spmd
from concourse.masks import make_identity

F32 = mybir.dt.float32
BF16 = mybir.dt.bfloat16
I16 = mybir.dt.int16
I32 = mybir.dt.int32

# ---------------------------------------------------------------- config
class CFG:
    N = 100000          # real nodes
    E = 1600000         # real edges
    H = 8               # heads
    CORES = 8
    NP = 100352         # padded nodes = 49*2048 = 784*128, divisible by 4 chunks
    CH = 25088          # chunk rows (int16-safe)
    TILE_N = 128        # nodes per tile
    SUB = 768           # slots per src-chunk sub-tile
    SUBS = 4
    TW = 64             # node-table row width (floats) = 256B
    NODE_CH = 2048      # nodes per node-phase trip

    @property
    def SLOTS(self):
        return self.SUB * self.SUBS

    @property
    def GROUPS(self):
        return self.SLOTS // 128

    @property
    def TILES(self):
        return self.NP // self.TILE_N

    @property
    def TPC(self):
        return self.TILES // self.CORES

    @property
    def NT(self):
        return self.NP // self.NODE_CH


cfg = CFG()


# ------------------------------------------------------------ host prep
def _sort_edges(cfg, src, dst):
    """dst-sorted tiling into regular 128-node tiles with src-chunk sub-tiles.
    Returns per-tile int16 idx arrays, dst_local, and slot map."""
    order = np.argsort(dst, kind="stable")
    src_s, dst_s = src[order], dst[order]
    tile_of = dst_s // cfg.TILE_N
    n_e = len(src_s)
    idxs = np.zeros((cfg.TILES, cfg.SUBS, cfg.SUB), np.int16)
    dloc = np.full((cfg.TILES, cfg.SLOTS), 255, np.int32)
    slot_of = np.empty(n_e, np.int64)   # edge (in sorted order) -> global slot
    counts = np.zeros((cfg.TILES, cfg.SUBS), np.int64)
    tstart = np.searchsorted(tile_of, np.arange(cfg.TILES + 1))
    for t in range(cfg.TILES):
        lo, hi = tstart[t], tstart[t + 1]
        if lo == hi:
            continue
        s, d = src_s[lo:hi], dst_s[lo:hi]
        ch = s // cfg.CH
        for c in range(cfg.SUBS):
            m = np.nonzero(ch == c)[0]
            k = len(m)
            assert k <= cfg.SUB, f"tile {t} chunk {c} overflow: {k}"
            idxs[t, c, :k] = (s[m] - c * cfg.CH).astype(np.int16)
            dloc[t, c * cfg.SUB + np.arange(k)] = d[m] - t * cfg.TILE_N
            slot_of[lo + m] = t * cfg.SLOTS + c * cfg.SUB + np.arange(k)
            counts[t, c] = k
    # wrap idx for dma_gather: j -> partition j%16, col j//16; replicate x8
    w = idxs.reshape(cfg.TILES, cfg.SUBS, cfg.SUB // 16, 16)
    w = np.transpose(w, (0, 3, 1, 2)).reshape(cfg.TILES, 16, cfg.SUBS * (cfg.SUB // 16))
    idx_w = np.tile(w, (1, 8, 1))                       # [TILES,128,SUBS*SUB/16]
    # dst_local arranged [TILES, 128, GROUPS]: slot = g*128+p
    dl = dloc.reshape(cfg.TILES, cfg.GROUPS, 128).transpose(0, 2, 1).copy()
    inv = np.empty(n_e, np.int64)
    inv[order] = np.arange(n_e)       # orig edge -> sorted pos
    return idx_w, dl, slot_of, inv


# ------------------------------------------------------------ gat launch
def build_gat(cfg, F_in, HF, F_out):
    """One GAT layer launch. Inputs per core:
       prevT_aug [F_in+1, NP] f32  (replicated; last row = ones)
       Waug f32 [F_in+1, F_in+16]  ([I+b | Wal | Wald] effective)
       Wmain bf16 [F_in, HF]
       idx   [TPC*128, SUBS*SUB/16] i16
       dloc  [TPC*128, GROUPS] i32
       out_slots [TPC*128, F_out] f32 (ExternalOutput)"""
    c = cfg
    F1 = F_in + 1
    TCOL = F_in + 16
    H = c.H
    FH = HF // H
    nc = bacc.Bacc("TRN2", target_bir_lowering=False, debug=False,
                   dynamic_dma_scratch_size=131072, num_swdge_queues=1)
    prevT = nc.dram_tensor("prevT", [F1, c.NP], F32, kind="ExternalInput")
    Waug = nc.dram_tensor("Waug", [F1, c.TW], F32, kind="ExternalInput")
    Wmain = nc.dram_tensor("Wmain", [F_in, HF], BF16, kind="ExternalInput")
    idx_t = nc.dram_tensor("idx", [c.TPC * 128, c.SUBS * (c.SUB // 16)], I16,
                           kind="ExternalInput")
    dloc_t = nc.dram_tensor("dloc", [c.TPC * 128, c.GROUPS], F32, kind="ExternalInput")
    gtbl = nc.dram_tensor("gtbl", [c.NP, c.TW], F32)     # internal
    out_t = nc.dram_tensor("out_slots", [c.TPC * 128, F_out], F32,
                           kind="ExternalOutput")

    with tile.TileContext(nc) as tc:
        with tc.tile_pool(name="const", bufs=1) as cpool:
            wa = cpool.tile([F1, c.TW], F32)
            nc.sync.dma_start(out=wa[:], in_=Waug[:])
            wm = cpool.tile([F_in, HF], BF16)
            nc.sync.dma_start(out=wm[:], in_=Wmain[:])
            iota = cpool.tile([128, 128], F32)
            nc.gpsimd.iota(iota[:], [[1, 128]], channel_multiplier=0,
                           allow_small_or_imprecise_dtypes=True)
            ident = cpool.tile([128, 128], F32)
            make_identity(nc, ident[:])
            identb = cpool.tile([128, 128], BF16)
            nc.vector.tensor_copy(out=identb[:], in_=ident[:])

            # ---------------- node phase: gtbl rows = prevT_aug.T @ Waug
            with tc.tile_pool(name="np_in", bufs=2) as pin, \
                 tc.tile_pool(name="np_out", bufs=2) as pout, \
                 tc.tile_pool(name="np_ps", bufs=2, space="PSUM") as pps:
                with tc.For_i(0, c.NT, 1) as i:
                    pv = pin.tile([F1, c.NODE_CH], F32)
                    nc.sync.dma_start(out=pv[:], in_=prevT[:, bass.ts(i, c.NODE_CH)])
                    ob = pout.tile([128, c.NODE_CH // 128, c.TW], F32)
                    for k in range(c.NODE_CH // 128):
                        ps = pps.tile([128, c.TW], F32, space="PSUM")
                        nc.tensor.matmul(out=ps[:], lhsT=pv[:, k * 128:(k + 1) * 128],
                                         rhs=wa[:], start=True, stop=True)
                        nc.scalar.copy(out=ob[:, k, :], in_=ps[:])
                    nc.sync.dma_start(
                        out=gtbl[bass.ts(i, c.NODE_CH), :].rearrange(
                            "(k p) w -> p k w", p=128),
                        in_=ob[:])
            tc.strict_bb_all_engine_barrier()

            # ---------------- edge phase
            pid = nc.sync.partition_id()
            with tc.tile_pool(name="eg", bufs=2) as pg, \
                 tc.tile_pool(name="es", bufs=2) as psb, \
                 tc.tile_pool(name="eps", bufs=1, space="PSUM") as pps, \
                 tc.tile_pool(name="eac", bufs=1, space="PSUM") as pac:
                with tc.For_i(0, c.TPC, 1) as i:
                    idxs = psb.tile([128, c.SUBS * (c.SUB // 16)], I16)
                    nc.sync.dma_start(out=idxs[:], in_=idx_t[bass.ts(i, 128), :])
                    dlc = psb.tile([128, c.GROUPS], F32)
                    nc.sync.dma_start(out=dlc[:], in_=dloc_t[bass.ts(i, 128), :])
                    adn = psb.tile([128, 8], F32)
                    nc.sync.dma_start(
                        out=adn[:],
                        in_=gtbl[bass.ds((pid * c.TPC + i) * 128, 128),
                                 F_in + 8:F_in + 16])
                    adn_bf = psb.tile([128, 8], BF16)
                    nc.scalar.copy(out=adn_bf[:], in_=adn[:])

                    gt = pg.tile([128, c.GROUPS, c.TW], F32)
                    spg = c.SUB // 128     # groups per sub-tile
                    for s in range(c.SUBS):
                        nc.gpsimd.dma_gather(
                            out_ap=gt[:, s * spg:(s + 1) * spg, :],
                            in_ap=gtbl[s * c.CH:(s + 1) * c.CH, :],
                            idxs_ap=idxs[:, s * (c.SUB // 16):(s + 1) * (c.SUB // 16)],
                            num_idxs=c.SUB, num_idxs_reg=c.SUB,
                            elem_size=c.TW, single_packet=False, queue_num=0)

                    vex = pg.tile([128, c.GROUPS, HF], BF16)
                    exb = psb.tile([128, c.GROUPS, H], BF16)
                    acc = pac.tile([128, HF], F32, space="PSUM")
                    den = pac.tile([128, H], F32, space="PSUM")
                    for g in range(c.GROUPS):
                        st = psb.tile([128, 128], BF16, tag="st")
                        nc.vector.tensor_scalar(
                            out=st[:], in0=iota[:], scalar1=dlc[:, g:g + 1],
                            scalar2=None, op0=mybir.AluOpType.is_equal)
                        # transpose prev cols -> expansion lhsT
                        tp = pps.tile([F_in, 128], F32, space="PSUM", tag="tp")
                        nc.tensor.transpose(out=tp[:], in_=gt[:, g, 0:F_in],
                                            identity=ident[:])
                        tpb = psb.tile([F_in, 128], BF16, tag="tpb")
                        nc.scalar.copy(out=tpb[:], in_=tp[:])
                        hp = pps.tile([128, HF], F32, space="PSUM", tag="hp")
                        nc.tensor.matmul(out=hp[:], lhsT=tpb[:], rhs=wm[:],
                                         start=True, stop=True)
                        # S transposed for al_d expansion
                        sg = pps.tile([128, 128], BF16, space="PSUM", tag="sg")
                        nc.tensor.transpose(out=sg[:], in_=st[:], identity=identb[:])
                        sgb = psb.tile([128, 128], BF16, tag="sgb")
                        nc.scalar.copy(out=sgb[:], in_=sg[:])
                        ep = pps.tile([128, H], F32, space="PSUM", tag="ep")
                        nc.tensor.matmul(out=ep[:], lhsT=sgb[:], rhs=adn_bf[:],
                                         start=True, stop=True)
                        # e = al_s + al_d ; leaky ; exp
                        ef = psb.tile([128, H], F32, tag="ef")
                        nc.vector.tensor_add(out=ef[:], in0=ep[:],
                                             in1=gt[:, g, F_in:F_in + 8])
                        eft = psb.tile([128, H], F32, tag="eft")
                        nc.vector.tensor_scalar(
                            out=eft[:], in0=ef[:], scalar1=0.2, scalar2=None,
                            op0=mybir.AluOpType.mult)
                        nc.vector.tensor_tensor(
                            out=ef[:], in0=ef[:], in1=eft[:],
                            op=mybir.AluOpType.max)
                        exf = psb.tile([128, H], F32, tag="exf")
                        nc.scalar.activation(out=exf[:], in_=ef[:],
                                             func=mybir.ActivationFunctionType.Exp)
                        nc.vector.tensor_copy(out=exb[:, g, :], in_=exf[:])
                        # weighted features
                        for h in range(H):
                            eng = nc.vector if h % 2 == 0 else nc.scalar
                            if h % 2 == 0:
                                nc.vector.tensor_scalar(
                                    out=vex[:, g, h * FH:(h + 1) * FH],
                                    in0=hp[:, h * FH:(h + 1) * FH],
                                    scalar1=exf[:, h:h + 1], scalar2=None,
                                    op0=mybir.AluOpType.mult)
                            else:
                                nc.scalar.activation(
                                    out=vex[:, g, h * FH:(h + 1) * FH],
                                    in_=hp[:, h * FH:(h + 1) * FH],
                                    func=mybir.ActivationFunctionType.Copy,
                                    scale=exf[:, h:h + 1])
                        nc.tensor.matmul(out=acc[:], lhsT=st[:], rhs=vex[:, g, :],
                                         start=(g == 0), stop=(g == c.GROUPS - 1))
                        nc.tensor.matmul(out=den[:], lhsT=st[:], rhs=exb[:, g, :],
                                         start=(g == 0), stop=(g == c.GROUPS - 1))
                    # epilogue: out = mean_h acc_h / den_h
                    dr = psb.tile([128, H], F32, tag="dr")
                    nc.vector.tensor_scalar(
                        out=dr[:], in0=den[:], scalar1=1e-30, scalar2=None,
                        op0=mybir.AluOpType.add)
                    nc.vector.reciprocal(out=dr[:], in_=dr[:])
                    nc.vector.tensor_scalar(out=dr[:], in0=dr[:], scalar1=1.0 / H,
                                            scalar2=None, op0=mybir.AluOpType.mult)
                    ot = psb.tile([128, H, F_out], F32, tag="ot")
                    for h in range(H):
                        if h % 2 == 0:
                            nc.vector.tensor_scalar(
                                out=ot[:, h, :], in0=acc[:, h * FH:(h + 1) * FH],
                                scalar1=dr[:, h:h + 1], scalar2=None,
                                op0=mybir.AluOpType.mult)
                        else:
                            nc.scalar.activation(
                                out=ot[:, h, :], in_=acc[:, h * FH:(h + 1) * FH],
                                func=mybir.ActivationFunctionType.Copy,
                                scale=dr[:, h:h + 1])
                    for step in [4, 2, 1]:
                        for h in range(step):
                            nc.vector.tensor_add(out=ot[:, h, :], in0=ot[:, h, :],
                                                 in1=ot[:, h + step, :])
                    nc.sync.dma_start(out=out_t[bass.ts(i, 128), :], in_=ot[:, 0, :])
    nc.compile()
    return nc


# ------------------------------------------------------------ mlp launch
def build_mlp(cfg):
    """Edge MLP launch. Tables U,V [NP,64] f32 built on device from out3T_aug;
       per tile: z = lrelu(U[src]+V[dst]+attr@Wc); out = sigm(lrelu(z@W2+b2)@W3+b3)."""
    c = cfg
    F1 = 65          # 64 + ones row
    nc = bacc.Bacc("TRN2", target_bir_lowering=False, debug=False,
                   dynamic_dma_scratch_size=131072, num_swdge_queues=1)
    prevT = nc.dram_tensor("prevT", [F1, c.NP], F32, kind="ExternalInput")
    Wuv = nc.dram_tensor("Wuv", [F1, 128], F32, kind="ExternalInput")  # [Wu | Wv]
    Wc = nc.dram_tensor("Wc", [16, 64], BF16, kind="ExternalInput")    # attr rows padded 10->16
    W2 = nc.dram_tensor("W2", [64, 16], BF16, kind="ExternalInput")
    b2 = nc.dram_tensor("b2", [16, 1], F32, kind="ExternalInput")
    W3 = nc.dram_tensor("W3", [16, 8], BF16, kind="ExternalInput")     # col0 real, rest 0
    bm3 = nc.dram_tensor("bm3", [1, 1], F32, kind="ExternalInput")
    idx_t = nc.dram_tensor("idx", [c.TPC * 128, c.SUBS * (c.SUB // 16)], I16,
                           kind="ExternalInput")
    dloc_t = nc.dram_tensor("dloc", [c.TPC * 128, c.GROUPS], F32, kind="ExternalInput")
    attr_t = nc.dram_tensor("attr", [c.TPC * 128, c.GROUPS * 16], BF16,
                            kind="ExternalInput")
    utbl = nc.dram_tensor("utbl", [c.NP, 64], F32)
    vtbl = nc.dram_tensor("vtbl", [c.NP, 64], F32)
    out_t = nc.dram_tensor("out_slots", [c.TPC, c.SLOTS], F32,
                           kind="ExternalOutput")

    with tile.TileContext(nc) as tc:
        with tc.tile_pool(name="const", bufs=1) as cpool:
            wuv = cpool.tile([F1, 128], F32)
            nc.sync.dma_start(out=wuv[:], in_=Wuv[:])
            wc = cpool.tile([16, 64], BF16)
            nc.sync.dma_start(out=wc[:], in_=Wc[:])
            w2 = cpool.tile([64, 16], BF16)
            nc.sync.dma_start(out=w2[:], in_=W2[:])
            b2s = cpool.tile([16, 1], F32)
            nc.sync.dma_start(out=b2s[:], in_=b2[:])
            w3 = cpool.tile([16, 8], BF16)
            nc.sync.dma_start(out=w3[:], in_=W3[:])
            b3s = cpool.tile([1, 1], F32)
            nc.sync.dma_start(out=b3s[:], in_=bm3[:])
            iota = cpool.tile([128, 128], F32)
            nc.gpsimd.iota(iota[:], [[1, 128]], channel_multiplier=0,
                           allow_small_or_imprecise_dtypes=True)
            ident = cpool.tile([128, 128], F32)
            make_identity(nc, ident[:])
            identb = cpool.tile([128, 128], BF16)
            nc.vector.tensor_copy(out=identb[:], in_=ident[:])

            # node phase -> U,V tables
            with tc.tile_pool(name="np_in", bufs=2) as pin, \
                 tc.tile_pool(name="np_out", bufs=2) as pout, \
                 tc.tile_pool(name="np_ps", bufs=2, space="PSUM") as pps:
                with tc.For_i(0, c.NT, 1) as i:
                    pv = pin.tile([F1, c.NODE_CH], F32)
                    nc.sync.dma_start(out=pv[:], in_=prevT[:, bass.ts(i, c.NODE_CH)])
                    ob = pout.tile([128, c.NODE_CH // 128, 128], F32)
                    for k in range(c.NODE_CH // 128):
                        ps = pps.tile([128, 128], F32, space="PSUM")
                        nc.tensor.matmul(out=ps[:], lhsT=pv[:, k * 128:(k + 1) * 128],
                                         rhs=wuv[:], start=True, stop=True)
                        nc.scalar.copy(out=ob[:, k, :], in_=ps[:])
                    nc.sync.dma_start(
                        out=utbl[bass.ts(i, c.NODE_CH), :].rearrange(
                            "(k p) w -> p k w", p=128),
                        in_=ob[:, :, 0:64])
                    nc.sync.dma_start(
                        out=vtbl[bass.ts(i, c.NODE_CH), :].rearrange(
                            "(k p) w -> p k w", p=128),
                        in_=ob[:, :, 64:128])
            tc.strict_bb_all_engine_barrier()

            pid = nc.sync.partition_id()
            with tc.tile_pool(name="eg", bufs=2) as pg, \
                 tc.tile_pool(name="es", bufs=2) as psb, \
                 tc.tile_pool(name="eps", bufs=1, space="PSUM") as pps:
                with tc.For_i(0, c.TPC, 1) as i:
                    idxs = psb.tile([128, c.SUBS * (c.SUB // 16)], I16)
                    nc.sync.dma_start(out=idxs[:], in_=idx_t[bass.ts(i, 128), :])
                    dlc = psb.tile([128, c.GROUPS], F32)
                    nc.sync.dma_start(out=dlc[:], in_=dloc_t[bass.ts(i, 128), :])
                    vnd = psb.tile([128, 64], F32)
                    nc.sync.dma_start(
                        out=vnd[:],
                        in_=vtbl[bass.ds((pid * c.TPC + i) * 128, 128), :])
                    vnd_bf = psb.tile([128, 64], BF16)
                    nc.scalar.copy(out=vnd_bf[:], in_=vnd[:])
                    att = pg.tile([128, c.GROUPS, 16], BF16, tag="att")
                    nc.sync.dma_start(out=att[:].rearrange("p g w -> p (g w)"),
                                      in_=attr_t[bass.ts(i, 128), :])

                    gt = pg.tile([128, c.GROUPS, 64], F32)
                    spg = c.SUB // 128
                    for s in range(c.SUBS):
                        nc.gpsimd.dma_gather(
                            out_ap=gt[:, s * spg:(s + 1) * spg, :],
                            in_ap=utbl[s * c.CH:(s + 1) * c.CH, :],
                            idxs_ap=idxs[:, s * (c.SUB // 16):(s + 1) * (c.SUB // 16)],
                            num_idxs=c.SUB, num_idxs_reg=c.SUB,
                            elem_size=64, single_packet=False, queue_num=0)

                    orow = psb.tile([1, c.GROUPS, 128], F32, tag="orow")
                    for g in range(c.GROUPS):
                        st = psb.tile([128, 128], BF16, tag="st")
                        nc.vector.tensor_scalar(
                            out=st[:], in0=iota[:], scalar1=dlc[:, g:g + 1],
                            scalar2=None, op0=mybir.AluOpType.is_equal)
                        sg = pps.tile([128, 128], BF16, space="PSUM", tag="sg")
                        nc.tensor.transpose(out=sg[:], in_=st[:], identity=identb[:])
                        sgb = psb.tile([128, 128], BF16, tag="sgb")
                        nc.scalar.copy(out=sgb[:], in_=sg[:])
                        # attr^T for attrW matmul
                        atp = pps.tile([16, 128], BF16, space="PSUM", tag="atp")
                        nc.tensor.transpose(out=atp[:], in_=att[:, g, :],
                                            identity=identb[:])
                        atpb = psb.tile([16, 128], BF16, tag="atpb")
                        nc.scalar.copy(out=atpb[:], in_=atp[:])
                        z1p = pps.tile([128, 64], F32, space="PSUM", tag="z1p")
                        nc.tensor.matmul(out=z1p[:], lhsT=atpb[:], rhs=wc[:],
                                         start=True, stop=False)
                        nc.tensor.matmul(out=z1p[:], lhsT=sgb[:], rhs=vnd_bf[:],
                                         start=False, stop=True)
                        z1 = psb.tile([128, 64], F32, tag="z1")
                        nc.vector.tensor_add(out=z1[:], in0=z1p[:], in1=gt[:, g, :])
                        z1s = psb.tile([128, 64], F32, tag="z1s")
                        nc.vector.tensor_scalar(
                            out=z1s[:], in0=z1[:], scalar1=0.12, scalar2=None,
                            op0=mybir.AluOpType.mult)
                        z1b = psb.tile([128, 64], BF16, tag="z1b")
                        nc.vector.tensor_tensor(
                            out=z1b[:], in0=z1[:], in1=z1s[:],
                            op=mybir.AluOpType.max)
                        z1t = pps.tile([64, 128], BF16, space="PSUM", tag="z1t")
                        nc.tensor.transpose(out=z1t[:], in_=z1b[:], identity=identb[:])
                        z1tb = psb.tile([64, 128], BF16, tag="z1tb")
                        nc.scalar.copy(out=z1tb[:], in_=z1t[:])
                        z2p = pps.tile([16, 128], F32, space="PSUM", tag="z2p")
                        nc.tensor.matmul(out=z2p[:], lhsT=w2[:], rhs=z1tb[:],
                                         start=True, stop=True)
                        z2f = psb.tile([16, 128], F32, tag="z2f")
                        nc.vector.tensor_scalar(
                            out=z2f[:], in0=z2p[:], scalar1=b2s[:, 0:1], scalar2=None,
                            op0=mybir.AluOpType.add)
                        z2s = psb.tile([16, 128], F32, tag="z2s")
                        nc.vector.tensor_scalar(
                            out=z2s[:], in0=z2f[:], scalar1=0.12, scalar2=None,
                            op0=mybir.AluOpType.mult)
                        z2b = psb.tile([16, 128], BF16, tag="z2b")
                        nc.vector.tensor_tensor(
                            out=z2b[:], in0=z2f[:], in1=z2s[:],
                            op=mybir.AluOpType.max)
                        z3p = pps.tile([8, 128], F32, space="PSUM", tag="z3p")
                        nc.tensor.matmul(out=z3p[:], lhsT=w3[:], rhs=z2b[:],
                                         start=True, stop=True)
                        nc.scalar.activation(out=orow[:, g, :], in_=z3p[0:1, :],
                                             func=mybir.ActivationFunctionType.Sigmoid,
                                             bias=b3s[:, 0:1])
                    nc.sync.dma_start(
                        out=out_t[bass.ts(i, 1), :],
                        in_=orow[:].rearrange("o g p -> o (g p)"))
    nc.compile()
    return nc


# ---------------------------------------------------------------- kernel
def _bf(x):
    return np.ascontiguousarray(x.astype(ml_dtypes.bfloat16))


def kernel(**inputs):
    c = cfg
    x = np.asarray(inputs["x"], np.float32)
    ei = np.asarray(inputs["edge_index"])
    ea = np.asarray(inputs["edge_attr"], np.float32)
    N, E, H = c.N, c.E, c.H
    cores = list(range(c.CORES))

    src, dst = ei[0].astype(np.int64), ei[1].astype(np.int64)
    loop = np.arange(N, dtype=np.int64)
    src_sl = np.concatenate([src, loop])
    dst_sl = np.concatenate([dst, loop])

    idx_g, dloc_g, _, _ = _sort_edges(c, src_sl, dst_sl)
    idx_m, dloc_m, slot_m, inv_m = _sort_edges(c, src, dst)

    def shard(a):   # [TILES*128, ...] -> per-core slices
        a = a.reshape(c.TILES * 128, -1)
        return [np.ascontiguousarray(a[cc * c.TPC * 128:(cc + 1) * c.TPC * 128])
                for cc in cores]

    idx_g_sh = shard(idx_g.reshape(c.TILES, 128, -1))
    dloc_g_sh = shard(dloc_g)
    idx_m_sh = shard(idx_m.reshape(c.TILES, 128, -1))
    dloc_m_sh = shard(dloc_m)

    # attr in slot space [TILES, SLOTS, 16] bf16, arranged [tiles,128,GROUPS*16]
    attr_slot = np.zeros((c.TILES * c.SLOTS, 16), np.float32)
    attr_slot[slot_m[inv_m[np.arange(E)]], :10] = ea  # slot of orig edge
    attr_slot = attr_slot.reshape(c.TILES, c.GROUPS, 128, 16).transpose(0, 2, 1, 3)
    attr_sh = [
        _bf(attr_slot[cc * c.TPC:(cc + 1) * c.TPC].reshape(c.TPC * 128, c.GROUPS * 16))
        for cc in cores]

    def prevT_aug(prev, bias):
        # [F, NP] + ones row; prev padded with zeros beyond N
        F = prev.shape[1]
        p = np.zeros((F + 1, c.NP), np.float32)
        p[:F, :N] = prev.T
        p[F, :] = 1.0
        return np.ascontiguousarray(p), bias

    def waug_eff(W, a_s, a_d, b_prev):
        # table row = [prev+b | al_s | al_d];  al = (prev+b) @ (W @ a^T-ish)
        Fin = W.shape[0]
        FHl = W.shape[1] // H
        Wal = np.einsum("ihf,hf->ih", W.reshape(Fin, H, FHl), a_s)
        Wad = np.einsum("ihf,hf->ih", W.reshape(Fin, H, FHl), a_d)
        wa = np.zeros((Fin + 1, 64), np.float32)
        wa[:Fin, :Fin] = np.eye(Fin, dtype=np.float32)
        wa[Fin, :Fin] = b_prev
        wa[:Fin, Fin:Fin + 8] = Wal
        wa[Fin, Fin:Fin + 8] = b_prev @ Wal
        wa[:Fin, Fin + 8:Fin + 16] = Wad
        wa[Fin, Fin + 8:Fin + 16] = b_prev @ Wad
        return wa

    layers = [
        (3, 128, 16, inputs["W1"], inputs["as1"], inputs["ad1"], np.zeros(3, np.float32), inputs["b1"]),
        (16, 256, 32, inputs["W2"], inputs["as2"], inputs["ad2"], np.asarray(inputs["b1"]), inputs["b2"]),
        (32, 512, 64, inputs["W3"], inputs["as3"], inputs["ad3"], np.asarray(inputs["b2"]), inputs["b3"]),
    ]

    prev = x
    b_carry = np.zeros(3, np.float32)
    for (F_in, HF, F_out, W, a_s, a_d, b_prev, b_out) in layers:
        W = np.asarray(W, np.float32)
        a_s = np.asarray(a_s, np.float32)
        a_d = np.asarray(a_d, np.float32)
        b_prev = np.asarray(b_prev, np.float32)
        nc = build_gat(c, F_in, HF, F_out)
        pT, _ = prevT_aug(prev, b_prev)
        wa = waug_eff(W, a_s, a_d, b_prev)
        wm = _bf(W)
        in_maps = []
        for cc in cores:
            in_maps.append({
                "prevT": pT, "Waug": wa, "Wmain": wm,
                "idx": idx_g_sh[cc], "dloc": dloc_g_sh[cc].astype(np.float32),
            })
        res = run_bass_kernel_spmd(nc, in_maps, cores)
        outs = np.concatenate([r["out_slots"] for r in res.results], 0)  # [NP, F_out]
        prev = outs[:N]
        b_carry = np.asarray(b_out, np.float32)

    out3 = prev            # [N, 64] raw (bias b3 not yet added)
    b3 = np.asarray(inputs["b3"], np.float32)
    Wm1 = np.asarray(inputs["Wm1"], np.float32)
    bm1 = np.asarray(inputs["bm1"], np.float32)
    Wm2 = np.asarray(inputs["Wm2"], np.float32)
    bm2 = np.asarray(inputs["bm2"], np.float32)
    Wm3 = np.asarray(inputs["Wm3"], np.float32)
    bm3 = np.asarray(inputs["bm3"], np.float32)

    Wu, Wv, Wc_ = Wm1[:64], Wm1[64:128], Wm1[128:138]
    pT, _ = prevT_aug(out3, None)
    wuv = np.zeros((65, 128), np.float32)
    wuv[:64, :64] = Wu
    wuv[64, :64] = b3 @ Wu + 0.5 * bm1
    wuv[:64, 64:] = Wv
    wuv[64, 64:] = b3 @ Wv + 0.5 * bm1
    wc16 = np.zeros((16, 64), np.float32)
    wc16[:10] = Wc_
    w3p = np.zeros((16, 8), np.float32)
    w3p[:, 0:1] = Wm3

    nc = build_mlp(c)
    in_maps = []
    for cc in cores:
        in_maps.append({
            "prevT": pT, "Wuv": wuv, "Wc": _bf(wc16), "W2": _bf(Wm2),
            "b2": bm2.reshape(16, 1), "W3": _bf(w3p), "bm3": bm3.reshape(1, 1),
            "idx": idx_m_sh[cc], "dloc": dloc_m_sh[cc].astype(np.float32),
            "attr": attr_sh[cc],
        })
    res = run_bass_kernel_spmd(nc, in_maps, cores)
    oslots = np.concatenate([r["out_slots"] for r in res.results], 0)  # [TILES, SLOTS]
    oslots = oslots.reshape(-1)
    out = oslots[slot_m[inv_m[np.arange(E)]]]
    return out.reshape(E, 1).astype(np.float32)



# revision 2
# speedup vs baseline: 21.4564x; 21.4564x over previous
"""Trainium2 Bass kernel for nn_BasicAttentionModel (3-layer GAT + edge MLP).

Fused single-launch design (8-core SPMD, dst-partitioned edges):
  - One Bacc kernel runs all 3 GAT layers + the edge MLP; node features
    never leave the device between layers.  Each layer's edge phase ends by
    computing the NEXT layer's full gather-table rows on the PE
    (row = (out+b) @ [I | Wal | Wad]) for this core's dst range; a 3.2MB
    AllGather then replicates the table to all cores.  4 AllGathers total.
  - Edges (with self-loops) are dst-sorted into 784 regular 128-node tiles,
    sub-tiled by src chunk so int16 gather indices stay in range; gather
    idx are uploaded 16-partition-wrapped (no 8x replication; replicated
    on device), dst_local as uint8.  attr rides in slot space at 10 bf16
    cols.  Total wire ~58MB H2D + ~10MB D2H (vs ~500MB for the 4-launch
    version), and one compile/load instead of four.
"""
import numpy as np
import ml_dtypes

import concourse.bacc as bacc
import concourse.bass as bass
import concourse.mybir as mybir
import concourse.tile as tile
from concourse.bass_utils import run_bass_kernel_spmd
from concourse.masks import make_identity

F32 = mybir.dt.float32
BF16 = mybir.dt.bfloat16
I16 = mybir.dt.int16
U8 = mybir.dt.uint8


# ---------------------------------------------------------------- config
class CFG:
    N = 100000          # real nodes
    E = 1600000         # real edges
    H = 8               # heads
    CORES = 8
    NP = 100352         # padded nodes = 784*128, divisible by 8*1792
    CH = 25088          # src chunk rows (int16-safe)
    TILE_N = 128
    SUB = 768           # slots per src-chunk sub-tile
    SUBS = 4
    TW = 64             # table row width (floats) = 256B
    NODE_CH = 1792      # nodes per phase-A trip (= RPC/7)

    SLOTS = SUB * SUBS              # 3072
    GROUPS = SLOTS // 128           # 24
    TILES = NP // TILE_N            # 784
    TPC = TILES // CORES            # 98
    RPC = TPC * TILE_N              # 12544 rows per core


cfg = CFG()


# ------------------------------------------------------------ host prep
def _sort_edges(c, src, dst):
    """dst-sorted tiling into regular 128-node tiles with src-chunk
    sub-tiles.  Returns 16-partition-wrapped int16 idx, uint8 dst_local
    arranged [TILES,128,GROUPS], and the orig-edge -> slot map."""
    n_e = len(src)
    key = (dst // c.TILE_N).astype(np.int64) * c.SUBS + src // c.CH
    order = np.argsort(key, kind="stable")
    key_s = key[order]
    # rank within each (tile, chunk) bucket
    bstart = np.searchsorted(key_s, np.arange(c.TILES * c.SUBS + 1))
    counts = np.diff(bstart)
    assert counts.max() <= c.SUB, f"bucket overflow: {counts.max()}"
    rank = np.arange(n_e) - np.repeat(bstart[:-1], counts)
    slot_sorted = key_s * c.SUB + rank          # global slot id
    idxs = np.zeros((c.TILES * c.SUBS, c.SUB), np.int16)
    dloc = np.full((c.TILES * c.SLOTS,), 255, np.uint8)
    idxs[key_s, rank] = (src[order] % c.CH).astype(np.int16)
    dloc[slot_sorted] = (dst[order] % c.TILE_N).astype(np.uint8)
    # wrap idx for dma_gather: j -> partition j%16, col j//16 (16 partitions)
    w = idxs.reshape(c.TILES, c.SUBS, c.SUB // 16, 16)
    idx_w = np.transpose(w, (0, 3, 1, 2)).reshape(c.TILES, 16, c.SUBS * (c.SUB // 16))
    dl = dloc.reshape(c.TILES, c.GROUPS, 128).transpose(0, 2, 1).copy()
    edge_slot = np.empty(n_e, np.int64)
    edge_slot[order] = slot_sorted              # slot of edge i (incl loops)
    return np.ascontiguousarray(idx_w), np.ascontiguousarray(dl), edge_slot


# ------------------------------------------------------------ the kernel
def build_fused(c):
    H = c.H
    IDXW = c.SUBS * (c.SUB // 16)       # 192
    nc = bacc.Bacc("TRN2", target_bir_lowering=False, debug=False,
                   dynamic_dma_scratch_size=131072, num_swdge_queues=1)

    # ---- external inputs (per core)
    xT_t = nc.dram_tensor("xT", [3, c.RPC], F32, kind="ExternalInput")
    wa1_t = nc.dram_tensor("wa1", [3, c.TW], F32, kind="ExternalInput")
    wa2_t = nc.dram_tensor("wa2", [16, c.TW], F32, kind="ExternalInput")
    wa3_t = nc.dram_tensor("wa3", [32, c.TW], F32, kind="ExternalInput")
    wm1_t = nc.dram_tensor("wm1", [3, 128], BF16, kind="ExternalInput")
    wm2_t = nc.dram_tensor("wm2", [16, 256], BF16, kind="ExternalInput")
    wm3_t = nc.dram_tensor("wm3", [32, 512], BF16, kind="ExternalInput")
    bb1_t = nc.dram_tensor("bb1", [128, 16], F32, kind="ExternalInput")
    bb2_t = nc.dram_tensor("bb2", [128, 32], F32, kind="ExternalInput")
    bb3_t = nc.dram_tensor("bb3", [128, 64], F32, kind="ExternalInput")
    bbm1_t = nc.dram_tensor("bbm1", [128, 64], F32, kind="ExternalInput")
    wu_t = nc.dram_tensor("wu", [64, 64], F32, kind="ExternalInput")
    wv_t = nc.dram_tensor("wv", [64, 64], F32, kind="ExternalInput")
    wc_t = nc.dram_tensor("wc", [10, 64], BF16, kind="ExternalInput")
    w2_t = nc.dram_tensor("w2", [64, 16], BF16, kind="ExternalInput")
    b2_t = nc.dram_tensor("b2", [16, 1], F32, kind="ExternalInput")
    w3_t = nc.dram_tensor("w3", [16, 8], BF16, kind="ExternalInput")
    bm3_t = nc.dram_tensor("bm3", [1, 1], F32, kind="ExternalInput")
    idx_t = nc.dram_tensor("idx", [c.TPC * 16, IDXW], I16, kind="ExternalInput")
    dl8_t = nc.dram_tensor("dl8", [c.RPC, c.GROUPS], U8, kind="ExternalInput")
    attr_t = nc.dram_tensor("attr", [c.RPC, c.GROUPS * 10], BF16,
                            kind="ExternalInput")
    out_t = nc.dram_tensor("out_slots", [c.TPC, c.SLOTS], F32,
                           kind="ExternalOutput")

    # ---- internal dram
    rows_t = nc.dram_tensor("rows", [c.RPC, c.TW], F32)     # per-core table rows
    vrows_t = nc.dram_tensor("vrows", [c.RPC, c.TW], F32)   # MLP V rows (local)
    gtbl = nc.dram_tensor("gtbl", [c.NP, c.TW], F32)        # gathered full table

    with tile.TileContext(nc) as tc:
        with tc.tile_pool(name="const", bufs=1) as cpool:
            def ctile(t, shape, dt):
                s = cpool.tile(shape, dt)
                nc.sync.dma_start(out=s[:], in_=t[:])
                return s
            wa1 = ctile(wa1_t, [3, c.TW], F32)
            wa2 = ctile(wa2_t, [16, c.TW], F32)
            wa3 = ctile(wa3_t, [32, c.TW], F32)
            wm1 = ctile(wm1_t, [3, 128], BF16)
            wm2 = ctile(wm2_t, [16, 256], BF16)
            wm3 = ctile(wm3_t, [32, 512], BF16)
            bb1 = ctile(bb1_t, [128, 16], F32)
            bb2 = ctile(bb2_t, [128, 32], F32)
            bb3 = ctile(bb3_t, [128, 64], F32)
            bbm1 = ctile(bbm1_t, [128, 64], F32)
            wu = ctile(wu_t, [64, 64], F32)
            wv = ctile(wv_t, [64, 64], F32)
            wc = ctile(wc_t, [10, 64], BF16)
            w2 = ctile(w2_t, [64, 16], BF16)
            b2s = ctile(b2_t, [16, 1], F32)
            w3 = ctile(w3_t, [16, 8], BF16)
            b3s = ctile(bm3_t, [1, 1], F32)
            iota = cpool.tile([128, 128], F32)
            nc.gpsimd.iota(iota[:], [[1, 128]], channel_multiplier=0,
                           allow_small_or_imprecise_dtypes=True)
            ident = cpool.tile([128, 128], F32)
            make_identity(nc, ident[:])
            identb = cpool.tile([128, 128], BF16)
            nc.vector.tensor_copy(out=identb[:], in_=ident[:])

            pid = nc.sync.partition_id()

            # ---------------- phase A: rows = xT-chunks @ wa1 (own range)
            with tc.tile_pool(name="pa_in", bufs=2) as pin, \
                 tc.tile_pool(name="pa_out", bufs=2) as pout, \
                 tc.tile_pool(name="pa_ps", bufs=2, space="PSUM") as pps:
                with tc.For_i(0, c.RPC // c.NODE_CH, 1) as j:
                    pv = pin.tile([3, c.NODE_CH], F32)
                    nc.sync.dma_start(out=pv[:], in_=xT_t[:, bass.ts(j, c.NODE_CH)])
                    ob = pout.tile([128, c.NODE_CH // 128, c.TW], F32)
                    for k in range(c.NODE_CH // 128):
                        ps = pps.tile([128, c.TW], F32, space="PSUM")
                        nc.tensor.matmul(out=ps[:], lhsT=pv[:, k * 128:(k + 1) * 128],
                                         rhs=wa1[:], start=True, stop=True)
                        nc.scalar.copy(out=ob[:, k, :], in_=ps[:])
                    nc.sync.dma_start(
                        out=rows_t[bass.ts(j, c.NODE_CH), :].rearrange(
                            "(k p) w -> p k w", p=128),
                        in_=ob[:])
            tc.strict_bb_all_engine_barrier()
            nc.gpsimd.collective_compute(
                "AllGather", mybir.AluOpType.bypass,
                replica_groups=[list(range(c.CORES))],
                ins=[rows_t[:]], outs=[gtbl[:]])
            tc.strict_bb_all_engine_barrier()

            # ---------------- GAT edge phases
            def edge_gat(F_in, HF, wm, bb, rows_next):
                """rows_next: list of (rhs_tile, dest_dram) to emit per tile."""
                FH = HF // H
                spg = c.SUB // 128
                with tc.tile_pool(name="eg", bufs=2) as pg, \
                     tc.tile_pool(name="es", bufs=2) as psb, \
                     tc.tile_pool(name="eps", bufs=1, space="PSUM") as pps, \
                     tc.tile_pool(name="eac", bufs=1, space="PSUM") as pac:
                    with tc.For_i(0, c.TPC, 1) as i:
                        idxs = psb.tile([128, IDXW], I16, tag="idxs")
                        for r in range(8):
                            nc.sync.dma_start(out=idxs[16 * r:16 * (r + 1), :],
                                              in_=idx_t[bass.ts(i, 16), :])
                        dl8 = psb.tile([128, c.GROUPS], U8, tag="dl8")
                        nc.sync.dma_start(out=dl8[:], in_=dl8_t[bass.ts(i, 128), :])
                        dlc = psb.tile([128, c.GROUPS], F32, tag="dlc")
                        nc.vector.tensor_copy(out=dlc[:], in_=dl8[:])
                        adn = psb.tile([128, 8], F32, tag="adn")
                        nc.sync.dma_start(
                            out=adn[:],
                            in_=gtbl[bass.ds((pid * c.TPC + i) * 128, 128),
                                     F_in + 8:F_in + 16])
                        adn_bf = psb.tile([128, 8], BF16, tag="adnb")
                        nc.scalar.copy(out=adn_bf[:], in_=adn[:])

                        gt = pg.tile([128, c.GROUPS, c.TW], F32)
                        for s in range(c.SUBS):
                            nc.gpsimd.dma_gather(
                                out_ap=gt[:, s * spg:(s + 1) * spg, :],
                                in_ap=gtbl[s * c.CH:(s + 1) * c.CH, :],
                                idxs_ap=idxs[:, s * (c.SUB // 16):(s + 1) * (c.SUB // 16)],
                                num_idxs=c.SUB, num_idxs_reg=c.SUB,
                                elem_size=c.TW, single_packet=False, queue_num=0)

                        vex = pg.tile([128, c.GROUPS, HF], BF16, tag="vex")
                        exb = psb.tile([128, c.GROUPS, H], BF16, tag="exb")
                        acc = pac.tile([128, HF], F32, space="PSUM")
                        den = pac.tile([128, H], F32, space="PSUM")
                        for g in range(c.GROUPS):
                            st = psb.tile([128, 128], BF16, tag="st")
                            nc.vector.tensor_scalar(
                                out=st[:], in0=iota[:], scalar1=dlc[:, g:g + 1],
                                scalar2=None, op0=mybir.AluOpType.is_equal)
                            tp = pps.tile([64, 128], F32, space="PSUM", tag="tp")
                            nc.tensor.transpose(out=tp[0:F_in, :], in_=gt[:, g, 0:F_in],
                                                identity=ident[:])
                            tpb = psb.tile([F_in, 128], BF16, tag="tpb")
                            nc.scalar.copy(out=tpb[:], in_=tp[0:F_in, :])
                            hp = pps.tile([128, HF], F32, space="PSUM", tag="hp")
                            nc.tensor.matmul(out=hp[:], lhsT=tpb[:], rhs=wm[:],
                                             start=True, stop=True)
                            sg = pps.tile([128, 128], BF16, space="PSUM", tag="sg")
                            nc.tensor.transpose(out=sg[:], in_=st[:], identity=identb[:])
                            sgb = psb.tile([128, 128], BF16, tag="sgb")
                            nc.scalar.copy(out=sgb[:], in_=sg[:])
                            ep = pps.tile([128, H], F32, space="PSUM", tag="ep")
                            nc.tensor.matmul(out=ep[:], lhsT=sgb[:], rhs=adn_bf[:],
                                             start=True, stop=True)
                            ef = psb.tile([128, H], F32, tag="ef")
                            nc.vector.tensor_add(out=ef[:], in0=ep[:],
                                                 in1=gt[:, g, F_in:F_in + 8])
                            eft = psb.tile([128, H], F32, tag="eft")
                            nc.vector.tensor_scalar(
                                out=eft[:], in0=ef[:], scalar1=0.2, scalar2=None,
                                op0=mybir.AluOpType.mult)
                            nc.vector.tensor_tensor(
                                out=ef[:], in0=ef[:], in1=eft[:],
                                op=mybir.AluOpType.max)
                            exf = psb.tile([128, H], F32, tag="exf")
                            nc.scalar.activation(out=exf[:], in_=ef[:],
                                                 func=mybir.ActivationFunctionType.Exp)
                            nc.vector.tensor_copy(out=exb[:, g, :], in_=exf[:])
                            for h in range(H):
                                if h % 2 == 0:
                                    nc.vector.tensor_scalar(
                                        out=vex[:, g, h * FH:(h + 1) * FH],
                                        in0=hp[:, h * FH:(h + 1) * FH],
                                        scalar1=exf[:, h:h + 1], scalar2=None,
                                        op0=mybir.AluOpType.mult)
                                else:
                                    nc.scalar.activation(
                                        out=vex[:, g, h * FH:(h + 1) * FH],
                                        in_=hp[:, h * FH:(h + 1) * FH],
                                        func=mybir.ActivationFunctionType.Copy,
                                        scale=exf[:, h:h + 1])
                            nc.tensor.matmul(out=acc[:], lhsT=st[:], rhs=vex[:, g, :],
                                             start=(g == 0), stop=(g == c.GROUPS - 1))
                            nc.tensor.matmul(out=den[:], lhsT=st[:], rhs=exb[:, g, :],
                                             start=(g == 0), stop=(g == c.GROUPS - 1))
                        # epilogue: ot = mean_h acc_h/den_h + b
                        F_out = FH
                        dr = psb.tile([128, H], F32, tag="dr")
                        nc.vector.tensor_scalar(
                            out=dr[:], in0=den[:], scalar1=1e-30, scalar2=None,
                            op0=mybir.AluOpType.add)
                        nc.vector.reciprocal(out=dr[:], in_=dr[:])
                        nc.vector.tensor_scalar(out=dr[:], in0=dr[:], scalar1=1.0 / H,
                                                scalar2=None, op0=mybir.AluOpType.mult)
                        ot = psb.tile([128, H, F_out], F32, tag="ot")
                        for h in range(H):
                            if h % 2 == 0:
                                nc.vector.tensor_scalar(
                                    out=ot[:, h, :], in0=acc[:, h * FH:(h + 1) * FH],
                                    scalar1=dr[:, h:h + 1], scalar2=None,
                                    op0=mybir.AluOpType.mult)
                            else:
                                nc.scalar.activation(
                                    out=ot[:, h, :], in_=acc[:, h * FH:(h + 1) * FH],
                                    func=mybir.ActivationFunctionType.Copy,
                                    scale=dr[:, h:h + 1])
                        for step in [4, 2, 1]:
                            for h in range(step):
                                nc.vector.tensor_add(out=ot[:, h, :], in0=ot[:, h, :],
                                                     in1=ot[:, h + step, :])
                        otb = psb.tile([128, F_out], F32, tag="otb")
                        nc.vector.tensor_add(out=otb[:], in0=ot[:, 0, :], in1=bb[:])
                        # next-layer table rows: (out+b) @ [I | Wal | Wad]
                        tp2 = pps.tile([64, 128], F32, space="PSUM", tag="tp")
                        nc.tensor.transpose(out=tp2[0:F_out, :], in_=otb[:],
                                            identity=ident[:])
                        tps = psb.tile([F_out, 128], F32, tag="tps")
                        nc.scalar.copy(out=tps[:], in_=tp2[0:F_out, :])
                        for rn, (rhs, dst_dram) in enumerate(rows_next):
                            rp = pps.tile([128, c.TW], F32, space="PSUM", tag="rp")
                            nc.tensor.matmul(out=rp[:], lhsT=tps[:], rhs=rhs[:],
                                             start=True, stop=True)
                            rs = psb.tile([128, c.TW], F32, tag=f"rs{rn}")
                            nc.scalar.copy(out=rs[:], in_=rp[:])
                            nc.sync.dma_start(out=dst_dram[bass.ts(i, 128), :],
                                              in_=rs[:])
                tc.strict_bb_all_engine_barrier()

            def gather_rows():
                nc.gpsimd.collective_compute(
                    "AllGather", mybir.AluOpType.bypass,
                    replica_groups=[list(range(c.CORES))],
                    ins=[rows_t[:]], outs=[gtbl[:]])
                tc.strict_bb_all_engine_barrier()

            edge_gat(3, 128, wm1, bb1, [(wa2, rows_t)])
            gather_rows()
            edge_gat(16, 256, wm2, bb2, [(wa3, rows_t)])
            gather_rows()
            edge_gat(32, 512, wm3, bb3, [(wu, rows_t), (wv, vrows_t)])
            gather_rows()

            # ---------------- MLP edge phase
            spg = c.SUB // 128
            with tc.tile_pool(name="mg", bufs=2) as pg, \
                 tc.tile_pool(name="ms", bufs=2) as psb, \
                 tc.tile_pool(name="mps", bufs=1, space="PSUM") as pps:
                with tc.For_i(0, c.TPC, 1) as i:
                    idxs = psb.tile([128, IDXW], I16, tag="idxs")
                    for r in range(8):
                        nc.sync.dma_start(out=idxs[16 * r:16 * (r + 1), :],
                                          in_=idx_t[bass.ts(i, 16), :])
                    dl8 = psb.tile([128, c.GROUPS], U8, tag="dl8")
                    nc.sync.dma_start(out=dl8[:], in_=dl8_t[bass.ts(i, 128), :])
                    dlc = psb.tile([128, c.GROUPS], F32, tag="dlc")
                    nc.vector.tensor_copy(out=dlc[:], in_=dl8[:])
                    vnd = psb.tile([128, 64], F32, tag="vnd")
                    nc.sync.dma_start(
                        out=vnd[:],
                        in_=vrows_t[bass.ds(i * 128, 128), :])
                    nc.vector.tensor_add(out=vnd[:], in0=vnd[:], in1=bbm1[:])
                    vnd_bf = psb.tile([128, 64], BF16, tag="vndb")
                    nc.scalar.copy(out=vnd_bf[:], in_=vnd[:])
                    att = pg.tile([128, c.GROUPS, 10], BF16, tag="att")
                    nc.sync.dma_start(out=att[:].rearrange("p g w -> p (g w)"),
                                      in_=attr_t[bass.ts(i, 128), :])

                    gt = pg.tile([128, c.GROUPS, c.TW], F32)
                    for s in range(c.SUBS):
                        nc.gpsimd.dma_gather(
                            out_ap=gt[:, s * spg:(s + 1) * spg, :],
                            in_ap=gtbl[s * c.CH:(s + 1) * c.CH, :],
                            idxs_ap=idxs[:, s * (c.SUB // 16):(s + 1) * (c.SUB // 16)],
                            num_idxs=c.SUB, num_idxs_reg=c.SUB,
                            elem_size=c.TW, single_packet=False, queue_num=0)

                    orow = psb.tile([1, c.GROUPS, 128], F32, tag="orow")
                    for g in range(c.GROUPS):
                        st = psb.tile([128, 128], BF16, tag="st")
                        nc.vector.tensor_scalar(
                            out=st[:], in0=iota[:], scalar1=dlc[:, g:g + 1],
                            scalar2=None, op0=mybir.AluOpType.is_equal)
                        sg = pps.tile([128, 128], BF16, space="PSUM", tag="sg")
                        nc.tensor.transpose(out=sg[:], in_=st[:], identity=identb[:])
                        sgb = psb.tile([128, 128], BF16, tag="sgb")
                        nc.scalar.copy(out=sgb[:], in_=sg[:])
                        atp = pps.tile([10, 128], BF16, space="PSUM", tag="atp")
                        nc.tensor.transpose(out=atp[:], in_=att[:, g, :],
                                            identity=identb[:])
                        atpb = psb.tile([10, 128], BF16, tag="atpb")
                        nc.scalar.copy(out=atpb[:], in_=atp[:])
                        z1p = pps.tile([128, 64], F32, space="PSUM", tag="z1p")
                        nc.tensor.matmul(out=z1p[:], lhsT=atpb[:], rhs=wc[:],
                                         start=True, stop=False)
                        nc.tensor.matmul(out=z1p[:], lhsT=sgb[:], rhs=vnd_bf[:],
                                         start=False, stop=True)
                        z1 = psb.tile([128, 64], F32, tag="z1")
                        nc.vector.tensor_add(out=z1[:], in0=z1p[:], in1=gt[:, g, :])
                        z1s = psb.tile([128, 64], F32, tag="z1s")
                        nc.vector.tensor_scalar(
                            out=z1s[:], in0=z1[:], scalar1=0.12, scalar2=None,
                            op0=mybir.AluOpType.mult)
                        z1b = psb.tile([128, 64], BF16, tag="z1b")
                        nc.vector.tensor_tensor(
                            out=z1b[:], in0=z1[:], in1=z1s[:],
                            op=mybir.AluOpType.max)
                        z1t = pps.tile([64, 128], BF16, space="PSUM", tag="z1t")
                        nc.tensor.transpose(out=z1t[:], in_=z1b[:], identity=identb[:])
                        z1tb = psb.tile([64, 128], BF16, tag="z1tb")
                        nc.scalar.copy(out=z1tb[:], in_=z1t[:])
                        z2p = pps.tile([16, 128], F32, space="PSUM", tag="z2p")
                        nc.tensor.matmul(out=z2p[:], lhsT=w2[:], rhs=z1tb[:],
                                         start=True, stop=True)
                        z2f = psb.tile([16, 128], F32, tag="z2f")
                        nc.vector.tensor_scalar(
                            out=z2f[:], in0=z2p[:], scalar1=b2s, scalar2=None,
                            op0=mybir.AluOpType.add)
                        z2s = psb.tile([16, 128], F32, tag="z2s")
                        nc.vector.tensor_scalar(
                            out=z2s[:], in0=z2f[:], scalar1=0.12, scalar2=None,
                            op0=mybir.AluOpType.mult)
                        z2b = psb.tile([16, 128], BF16, tag="z2b")
                        nc.vector.tensor_tensor(
                            out=z2b[:], in0=z2f[:], in1=z2s[:],
                            op=mybir.AluOpType.max)
                        z3p = pps.tile([8, 128], F32, space="PSUM", tag="z3p")
                        nc.tensor.matmul(out=z3p[:], lhsT=w3[:], rhs=z2b[:],
                                         start=True, stop=True)
                        nc.scalar.activation(out=orow[:, g, :], in_=z3p[0:1, :],
                                             func=mybir.ActivationFunctionType.Sigmoid,
                                             bias=b3s)
                    nc.sync.dma_start(
                        out=out_t[bass.ts(i, 1), :],
                        in_=orow[:].rearrange("o g p -> o (g p)"))
    nc.compile()
    return nc


# ---------------------------------------------------------------- driver
def _bf(x):
    return np.ascontiguousarray(x.astype(ml_dtypes.bfloat16))


def kernel(**inputs):
    c = cfg
    H = c.H
    x = np.asarray(inputs["x"], np.float32)
    ei = np.asarray(inputs["edge_index"])
    ea = np.asarray(inputs["edge_attr"], np.float32)
    cores = list(range(c.CORES))

    src, dst = ei[0].astype(np.int64), ei[1].astype(np.int64)
    loop = np.arange(c.N, dtype=np.int64)
    src_sl = np.concatenate([src, loop])
    dst_sl = np.concatenate([dst, loop])
    idx_w, dl, edge_slot = _sort_edges(c, src_sl, dst_sl)

    # attr in slot space, 10 bf16 cols
    attr_slot = np.zeros((c.TILES * c.SLOTS, 10), np.float32)
    attr_slot[edge_slot[:c.E]] = ea
    attr_slot = attr_slot.reshape(c.TILES, c.GROUPS, 128, 10).transpose(0, 2, 1, 3)
    attr_slot = _bf(attr_slot.reshape(c.TILES * 128, c.GROUPS * 10))

    # xT padded [3, NP]
    xT = np.zeros((3, c.NP), np.float32)
    xT[:, :c.N] = x.T

    def wal_pair(W, a_s, a_d):
        Fin = W.shape[0]
        FH = W.shape[1] // H
        Wal = np.einsum("ihf,hf->ih", W.reshape(Fin, H, FH), a_s)
        Wad = np.einsum("ihf,hf->ih", W.reshape(Fin, H, FH), a_d)
        wa = np.zeros((Fin, c.TW), np.float32)
        wa[:, :Fin] = np.eye(Fin, dtype=np.float32)
        wa[:, Fin:Fin + 8] = Wal
        wa[:, Fin + 8:Fin + 16] = Wad
        return wa

    W1 = np.asarray(inputs["W1"], np.float32)
    W2 = np.asarray(inputs["W2"], np.float32)
    W3 = np.asarray(inputs["W3"], np.float32)
    wa1 = wal_pair(W1, np.asarray(inputs["as1"], np.float32),
                   np.asarray(inputs["ad1"], np.float32))
    wa2 = wal_pair(W2, np.asarray(inputs["as2"], np.float32),
                   np.asarray(inputs["ad2"], np.float32))
    wa3 = wal_pair(W3, np.asarray(inputs["as3"], np.float32),
                   np.asarray(inputs["ad3"], np.float32))
    b1 = np.asarray(inputs["b1"], np.float32)
    b2 = np.asarray(inputs["b2"], np.float32)
    b3 = np.asarray(inputs["b3"], np.float32)
    Wm1 = np.asarray(inputs["Wm1"], np.float32)
    bm1 = np.asarray(inputs["bm1"], np.float32)
    Wm2 = np.asarray(inputs["Wm2"], np.float32)
    bm2 = np.asarray(inputs["bm2"], np.float32)
    Wm3 = np.asarray(inputs["Wm3"], np.float32)
    bm3 = np.asarray(inputs["bm3"], np.float32)

    w3p = np.zeros((16, 8), np.float32)
    w3p[:, 0:1] = Wm3

    nc = build_fused(c)
    in_maps = []
    for cc in cores:
        in_maps.append({
            "xT": np.ascontiguousarray(xT[:, cc * c.RPC:(cc + 1) * c.RPC]),
            "wa1": wa1, "wa2": wa2, "wa3": wa3,
            "wm1": _bf(W1), "wm2": _bf(W2), "wm3": _bf(W3),
            "bb1": np.tile(b1, (128, 1)), "bb2": np.tile(b2, (128, 1)),
            "bb3": np.tile(b3, (128, 1)), "bbm1": np.tile(bm1, (128, 1)),
            "wu": Wm1[:64], "wv": Wm1[64:128], "wc": _bf(Wm1[128:138]),
            "w2": _bf(Wm2), "b2": bm2.reshape(16, 1),
            "w3": _bf(w3p), "bm3": bm3.reshape(1, 1),
            "idx": idx_w.reshape(c.TILES * 16, -1)[cc * c.TPC * 16:(cc + 1) * c.TPC * 16],
            "dl8": dl.reshape(c.TILES * 128, -1)[cc * c.RPC:(cc + 1) * c.RPC],
            "attr": attr_slot[cc * c.RPC:(cc + 1) * c.RPC],
        })
    res = run_bass_kernel_spmd(nc, in_maps, cores)
    oslots = np.concatenate([r["out_slots"] for r in res.results], 0).reshape(-1)
    out = oslots[edge_slot[:c.E]]
    return out.reshape(c.E, 1).astype(np.float32)
